# revision 4
# baseline (speedup 1.0000x reference)
"""Cached single-head attention (B=4, QLEN=PAST=2048, D=2048) on 8 Trainium2
NeuronCores — fp8-DoubleRow variant (final).

Sharding: identical to baseline (core (b, h) owns 2048 KV positions: past half
h plus the new keys projected from query half h; 2-core AllGather exchanges
Q^T halves; host combines partial softmax numer/denom).

New in v2: the attention-phase matmuls use fp8 DoubleRow (2 MACs/cell/cycle)
where precision allows:
  - scores vs NEW KV (projected k, sigma~0.58) in fp8-DR; scores vs PAST KV
    (raw N(0,1) k) stay bf16.
  - P@V: new-KV chunks in fp8-DR with mean-centered P (dp = p - c) and an
    exact host-side rank-1 correction c * colsum(v); past-KV chunks bf16.
The centering suppresses fp8 quantization error ~3x and the exact correction
removes the common-mode error of fp8(v).
"""

import sys

sys.path.insert(0, "/opt/trn_rl_repo")

import numpy as np
import ml_dtypes

import concourse.bacc as bacc
import concourse.mybir as mybir
import concourse.tile as tile
from concourse.bass_utils import run_bass_kernel_spmd
from concourse.tile_rust import add_dep_helper

BF16 = mybir.dt.bfloat16
F32 = mybir.dt.float32
DR = mybir.MatmulPerfMode.DoubleRow

# fp8 flavor for the DoubleRow matmuls ('e4' always legal; 'e3' pending HW
# validation of the 4-bit mantissa surviving the DR datapath)
F8_KIND = "e4"
F8 = mybir.dt.float8e4 if F8_KIND == "e4" else mybir.dt.float8e3
F8NP = ml_dtypes.float8_e4m3 if F8_KIND == "e4" else ml_dtypes.float8_e3m4
if F8_KIND == "e3":
    mybir.MATMUL_PERF_MODE_DTYPES = tuple(mybir.MATMUL_PERF_MODE_DTYPES) + (F8,)

B = 4
T = 2048  # QLEN == PAST
D = 2048
P = 128
H = T // 2  # query/kv half owned by one core
DC = D // P  # 16 contraction chunks
EC = D // P  # 16 e-chunks
KC = 16  # kv chunks of 128 (2048 kv positions per core)
PKC = 8  # past kv chunks (0..7); new are 8..15
QBS = 512  # q block size
NQB = T // QBS  # 4 q blocks
WEB = 256  # weight tile e-block width
NWB = D // WEB  # 8 weight tiles per W
SCALE = 1.0 / float(np.sqrt(D))
# centering constants for P = exp(scale * scores): E[p] per kv half
C_PAST = 1.33
C_NEW = 1.06
# clamp so dp fits fp8 range (needed for e3m4 max 15.5; harmless for e4m3)
DP_CLAMP = 15.0 if F8_KIND == "e3" else 200.0

_NC_CACHE: dict = {}


def build_nc():
    nc = bacc.Bacc()
    xa = nc.dram_tensor("xa", [P, DC, QBS], BF16, kind="ExternalInput")
    xb = nc.dram_tensor("xb", [P, DC, QBS], BF16, kind="ExternalInput")
    wq = nc.dram_tensor("wq", [NWB, P, DC, WEB], BF16, kind="ExternalInput")
    wk = nc.dram_tensor("wk", [NWB, P, DC, WEB], BF16, kind="ExternalInput")
    xa8 = nc.dram_tensor("xa8", [P, DC, QBS], F8, kind="ExternalInput")
    xb8 = nc.dram_tensor("xb8", [P, DC, QBS], F8, kind="ExternalInput")
    wv8 = nc.dram_tensor("wv8", [NWB, P, DC, WEB], F8, kind="ExternalInput")
    pk = nc.dram_tensor("pk", [P, EC, H], BF16, kind="ExternalInput")
    pv = nc.dram_tensor("pv", [P, PKC, D], BF16, kind="ExternalInput")
    numer = nc.dram_tensor("numer", [T, D], F32, kind="ExternalOutput")
    denom = nc.dram_tensor("denom", [1, T], F32, kind="ExternalOutput")

    with tile.TileContext(nc) as tc:
        _emit(nc, tc, xa, xb, xa8, xb8, wq, wk, wv8, pk, pv, numer, denom)
    nc.finalize()
    return nc


def _emit(nc, tc, xa_d, xb_d, xa8_d, xb8_d, wq_d, wk_d, wv8_d, pk_d, pv_d, numer, denom):
    with (
        tc.tile_pool(name="res", bufs=1) as res,
        tc.tile_pool(name="dram", bufs=1, space="DRAM") as dram,
    ):
        # Past K^T bf16: kt_bf[p, ec, kv] (kv 0:1024). New K^T fp8: kt8[p, ec, n]
        # (new kv 0:1024 -> chunks 8..15 of the core's kv range).
        kt_bf = res.tile([P, EC, H], BF16)
        kt8 = res.tile([P, EC, H], F8)
        # V: past chunks bf16 v_bf[p, c, e] (c=0..7), halves in fp8 v8[p, c, e]
        # covering the kv chunks that run through the fp8 PV path.  Past fp8
        # copy comes from host (pv8), new half is cast from the V-proj PSUM.
        v_bf = res.tile([P, PKC, D], BF16)
        v8 = res.tile([P, KC - PKC, D], F8)
        qt0 = res.tile([P, EC, QBS], BF16)  # persistent qb=0 prefetch
        ones = nc.const_aps.tensor(1.0, (P, 1), BF16)
        # DoubleRow weight APs need the plane-pair step to be a multiple of
        # 16B; pad the ones column out to 16 and slice [:, :, 0:1].
        ones8 = res.tile([P, 2, 16], F8)
        nc.vector.memset(ones8[:], 1.0)
        qtd_own = dram.tile([H // QBS, EC, P, QBS], BF16)
        qtd_full = dram.tile([2, H // QBS, EC, P, QBS], BF16)

        # ---- PE warmup: dummy matmuls with no data deps run during the
        # initial DMA wait so the HAM clock gate reaches 8/8 before the first
        # real matmul (otherwise the first ~3.4us of matmuls run at 1.2 GHz).
        with (
            tc.tile_pool(name="wup", bufs=1) as wup_pool,
            tc.tile_pool(name="wps", bufs=1, space="PSUM") as wps_pool,
        ):
            garbage = wup_pool.tile([P, QBS], BF16)
            nc.vector.memset(garbage[:], 0.0)
            wps = wps_pool.tile([P, QBS], F32, tag="warm")
            for _ in range(22):
                nc.tensor.matmul(wps[0:1, :], ones[:], garbage[:], start=True, stop=True)

        # ---- prologue: projections (bf16, unchanged from baseline) ----
        with (
            tc.tile_pool(name="xhp", bufs=1) as xh_pool,
            tc.tile_pool(name="w", bufs=3) as w_pool,
            tc.tile_pool(name="qstage", bufs=2) as qstage,
            tc.tile_pool(name="pps", bufs=4, space="PSUM") as pps,
        ):
            xh = xh_pool.tile([P, 2, DC, QBS], BF16, tag="xh")
            xh8 = xh_pool.tile([P, 2, DC, QBS], F8, tag="xh8")
            wv0 = xh_pool.tile([P, DC, WEB], F8, tag="wv0")

            anchor = None
            for eb in range(NWB):
                wq = w_pool.tile([P, DC, WEB], BF16, tag="w")
                nc.sync.dma_start(wq[:], wq_d[eb])
                if eb == 0:
                    nc.sync.dma_start(xh[:, 0, 0 : DC // 2], xa_d[:, 0 : DC // 2])
                    nc.sync.dma_start(xh[:, 0, DC // 2 : DC], xa_d[:, DC // 2 : DC])
                    nc.sync.dma_start(xh[:, 1], xb_d[:])
                for es in range(WEB // P):
                    ec = (eb * WEB) // P + es
                    for qb in range(H // QBS):
                        ps = pps.tile([P, QBS], F32, tag="proj")
                        for dc in range(DC):
                            nc.tensor.matmul(
                                ps[:],
                                wq[:, dc, es * P : (es + 1) * P],
                                xh[:, qb, dc, :],
                                start=(dc == 0),
                                stop=(dc == DC - 1),
                            )
                        qs = qstage.tile([P, QBS], BF16, tag="qs")
                        cp = nc.vector.tensor_copy(qs[:], ps[:])
                        if anchor is None:
                            anchor = cp
                        nc.sync.dma_start(qtd_own[qb, ec], qs[:])

            # bulk past-K/V loads, gated behind the first Q^T tile
            first = None
            for c in range(NWB):
                ktd = nc.sync.dma_start(
                    kt_bf[:, 2 * c : 2 * c + 2, :], pk_d[:, 2 * c : 2 * c + 2, :]
                )
                if first is None:
                    first = ktd
                nc.sync.dma_start(v_bf[:, c, :], pv_d[:, c, :])
            add_dep_helper(anchor.ins, first.ins, reason="delay bulk past load")
            nc.sync.dma_start(xh8[:, 0], xa8_d[:])
            nc.sync.dma_start(xh8[:, 1], xb8_d[:])
            nc.sync.dma_start(wv0[:], wv8_d[0])

            nc.gpsimd.collective_compute(
                "AllGather",
                mybir.AluOpType.bypass,
                replica_groups=[[0, 1], [2, 3], [4, 5], [6, 7]],
                ins=[qtd_own.opt()],
                outs=[qtd_full.opt()],
            )

            # K_new^T -> kt8 (fp8)
            for eb in range(NWB):
                wk = w_pool.tile([P, DC, WEB], BF16, tag="w")
                nc.sync.dma_start(wk[:], wk_d[eb])
                for es in range(WEB // P):
                    ec = (eb * WEB) // P + es
                    for nb in range(H // QBS):
                        ps = pps.tile([P, QBS], F32, tag="proj")
                        for dc in range(DC):
                            nc.tensor.matmul(
                                ps[:],
                                wk[:, dc, es * P : (es + 1) * P],
                                xh[:, nb, dc, :],
                                start=(dc == 0),
                                stop=(dc == DC - 1),
                            )
                        nc.vector.tensor_copy(
                            kt8[:, ec, nb * QBS : (nb + 1) * QBS], ps[:]
                        )
            # V_new -> v8: fp8-DR (x8 @ (16*Wv)8), copy-out scales by 1/16.
            # v errors are suppressed by the centered-PV host correction.
            for eb in range(NWB):
                if eb == 0:
                    wv = wv0
                else:
                    wv = w_pool.tile([P, DC, WEB], F8, tag="w8")
                    nc.sync.dma_start(wv[:], wv8_d[eb])
                for tch in range(H // P):
                    hf, ts_ = divmod(tch, QBS // P)
                    ps = pps.tile([P, WEB], F32, tag="proj")
                    for dc2 in range(DC // 2):
                        nc.tensor.matmul(
                            ps[:],
                            xh8[:, hf, 2 * dc2 : 2 * dc2 + 2, ts_ * P : (ts_ + 1) * P],
                            wv[:, 2 * dc2 : 2 * dc2 + 2, :],
                            start=(dc2 == 0),
                            stop=(dc2 == DC // 2 - 1),
                            perf_mode=DR,
                        )
                    nc.vector.tensor_scalar(
                        v8[:, (KC - PKC - H // P) + tch, eb * WEB : (eb + 1) * WEB],
                        ps[:],
                        1.0 / 16.0,
                        None,
                        mybir.AluOpType.mult,
                    )
            nc.sync.dma_start(qt0[:], qtd_full[0, 0].rearrange("ec p q -> p ec q"))

        # ---- attention ----
        with (
            tc.tile_pool(name="res2", bufs=1) as res2,
            tc.tile_pool(name="qt", bufs=2) as qt_pool,
            tc.tile_pool(name="qt8p", bufs=2) as qt8_pool,
            tc.tile_pool(name="pt", bufs=1) as pt_pool,
            tc.tile_pool(name="pex", bufs=3) as pex_pool,
            tc.tile_pool(name="ostage", bufs=2) as ostage,
            tc.tile_pool(name="sps", bufs=2, space="PSUM") as sps,
            tc.tile_pool(name="ops", bufs=3, space="PSUM") as ops,
            tc.tile_pool(name="dps", bufs=2, space="PSUM") as dps,
        ):
            denom_sb = res2.tile([1, T], F32, name="denom_sb")
            for qb in range(NQB):
                rank, sub = divmod(qb, NQB // 2)
                if qb == 0:
                    qt = qt0
                else:
                    qt = qt_pool.tile([P, EC, QBS], BF16, tag="qt")
                    nc.sync.dma_start(
                        qt[:], qtd_full[rank, sub].rearrange("ec p q -> p ec q")
                    )
                qt8 = qt8_pool.tile([P, EC, QBS], F8, tag="qt8")
                nc.vector.tensor_copy(qt8[:], qt[:])
                # pt_bf: exp(scores) bf16 for past chunks; dpt8: fp8 centered
                # for new chunks
                pt_bf = pt_pool.tile([P, PKC, QBS], BF16, tag="ptbf")
                dpt8 = pt_pool.tile([P, KC - PKC, QBS], F8, tag="dpt8")
                for kc in range(KC):
                    ps = sps.tile([P, QBS], F32, tag="s")
                    if kc < PKC:
                        for ec in range(EC):
                            nc.tensor.matmul(
                                ps[:],
                                kt_bf[:, ec, kc * P : (kc + 1) * P],
                                qt[:, ec, :],
                                start=(ec == 0),
                                stop=(ec == EC - 1),
                            )
                        nc.scalar.activation(
                            pt_bf[:, kc, :],
                            ps[:],
                            mybir.ActivationFunctionType.Exp,
                            scale=SCALE,
                        )
                    else:
                        for ec2 in range(EC // 2):
                            nc.tensor.matmul(
                                ps[:],
                                kt8[:, 2 * ec2 : 2 * ec2 + 2, (kc - PKC) * P : (kc - PKC + 1) * P],
                                qt8[:, 2 * ec2 : 2 * ec2 + 2, :],
                                start=(ec2 == 0),
                                stop=(ec2 == EC // 2 - 1),
                                perf_mode=DR,
                            )
                        pex = pex_pool.tile([P, QBS], F32, tag="pex")
                        nc.scalar.activation(
                            pex[:],
                            ps[:],
                            mybir.ActivationFunctionType.Exp,
                            scale=SCALE,
                        )
                        nc.vector.tensor_scalar(
                            dpt8[:, kc - PKC, :],
                            pex[:],
                            C_NEW,
                            DP_CLAMP,
                            mybir.AluOpType.subtract,
                            mybir.AluOpType.min,
                        )
                # denom: past via ones@pt_bf (M=1), new via fp8-DR ones
                pd = dps.tile([P, QBS], F32, tag="d")
                for kc in range(PKC):
                    nc.tensor.matmul(
                        pd[0:1, :],
                        ones[:],
                        pt_bf[:, kc, :],
                        start=(kc == 0),
                        stop=False,
                    )
                for kc2 in range((KC - PKC) // 2):
                    nc.tensor.matmul(
                        pd[0:1, :],
                        ones8[:, :, 0:1],
                        dpt8[:, 2 * kc2 : 2 * kc2 + 2, :],
                        start=False,
                        stop=(kc2 == (KC - PKC) // 2 - 1),
                        perf_mode=DR,
                    )
                nc.vector.tensor_copy(
                    denom_sb[:, qb * QBS : (qb + 1) * QBS], pd[0:1, :]
                )
                nc.sync.dma_start(
                    denom[:, qb * QBS : (qb + 1) * QBS],
                    denom_sb[:, qb * QBS : (qb + 1) * QBS],
                )
                # numer[q, e]: past chunks bf16 + new chunks fp8-DR, one PSUM
                # accumulation group per (qc, eb)
                for qc in range(QBS // P):
                    qrow = qb * (QBS // P) + qc
                    for eb in range(D // QBS):
                        po = ops.tile([P, QBS], F32, tag="o")
                        for kc in range(PKC):
                            nc.tensor.matmul(
                                po[:],
                                pt_bf[:, kc, qc * P : (qc + 1) * P],
                                v_bf[:, kc, eb * QBS : (eb + 1) * QBS],
                                start=(kc == 0),
                                stop=False,
                            )
                        nkc = (KC - PKC) // 2
                        for kc2 in range(nkc):
                            nc.tensor.matmul(
                                po[:],
                                dpt8[:, 2 * kc2 : 2 * kc2 + 2, qc * P : (qc + 1) * P],
                                v8[:, 2 * kc2 : 2 * kc2 + 2, eb * QBS : (eb + 1) * QBS],
                                start=False,
                                stop=(kc2 == nkc - 1),
                                perf_mode=DR,
                            )
                        ost = ostage.tile([P, QBS], F32, tag="ost")
                        nc.vector.tensor_copy(ost[:], po[:])
                        nc.sync.dma_start(
                            numer[
                                qrow * P : (qrow + 1) * P,
                                eb * QBS : (eb + 1) * QBS,
                            ],
                            ost[:],
                        )


def _get_nc():
    if "nc" not in _NC_CACHE:
        _NC_CACHE["nc"] = build_nc()
    return _NC_CACHE["nc"]


def _pack_w(W, bf):
    return np.ascontiguousarray(
        np.asarray(W).reshape(NWB, WEB, DC, P).transpose(0, 3, 2, 1)
    ).astype(bf)


def make_in_maps(x, past_k, past_v, Wq, Wk, Wv):
    bf = ml_dtypes.bfloat16
    wq = _pack_w(Wq, bf)
    wk = _pack_w(Wk, bf)
    wv8 = _pack_w(np.asarray(Wv) * np.float32(16.0), F8NP)
    in_maps = []
    for b in range(B):
        for h in range(2):
            sel = slice(H * h, H * (h + 1))
            xs = np.asarray(x[b, sel])  # [H, D]
            xap = np.ascontiguousarray(
                xs[0:QBS].reshape(QBS, DC, P).transpose(2, 1, 0)
            )
            xbp0 = np.ascontiguousarray(
                xs[QBS:H].reshape(QBS, DC, P).transpose(2, 1, 0)
            )
            xa = xap.astype(bf)
            xbp = xbp0.astype(bf)
            xa8p = xap.astype(F8NP)
            xb8p = xbp0.astype(F8NP)
            pk = np.ascontiguousarray(
                np.asarray(past_k[b, sel]).reshape(H, EC, P).transpose(2, 1, 0)
            ).astype(bf)
            # past V chunks: [p, c, e] = past_v[b, hH + c*P + p, e]
            pvr = np.asarray(past_v[b, sel]).reshape(H // P, P, D).transpose(1, 0, 2)
            pvp = np.ascontiguousarray(pvr[:, :PKC]).astype(bf)
            in_maps.append(
                {
                    "xa": xa,
                    "xb": xbp,
                    "xa8": xa8p,
                    "xb8": xb8p,
                    "wq": wq,
                    "wk": wk,
                    "wv8": wv8,
                    "pk": pk,
                    "pv": pvp,
                }
            )
    return in_maps


def combine(results, corr_num, corr_den):
    out = np.empty((B, T, D), dtype=np.float32)
    for b in range(B):
        r0, r1 = results[2 * b], results[2 * b + 1]
        num = r0["numer"].astype(np.float64) + r1["numer"] + corr_num[b]
        den = (
            r0["denom"].astype(np.float64) + r1["denom"]
        ).reshape(T) + corr_den
        out[b] = (num / den[:, None]).astype(np.float32)
    return np.round(out, 4)


def host_corrections(x, past_v, Wv):
    """c * colsum(v) over the fp8 PV chunks (new half of both cores), exact."""
    x64 = np.asarray(x, dtype=np.float64)
    Wv64 = np.asarray(Wv, dtype=np.float64)
    corr_num = np.empty((B, D), np.float64)
    for b in range(B):
        xsum = x64[b].sum(axis=0)  # both halves' new v
        corr_num[b] = C_NEW * (xsum @ Wv64.T)
    corr_den = C_NEW * (2 * H)  # 1024 new kv per core, 2 cores per batch
    return corr_num, corr_den


def kernel(x, past_k, past_v, Wq, Wk, Wv, _trace=False, _trace_cores=None):
    nc = _get_nc()
    in_maps = make_in_maps(x, past_k, past_v, Wq, Wk, Wv)
    corr_num, corr_den = host_corrections(x, past_v, Wv)
    res = run_bass_kernel_spmd(
        nc,
        in_maps,
        list(range(8)),
        trace=_trace,
        trace_cores=_trace_cores,
    )
    out = combine(res.results, corr_num, corr_den)
    kernel.last_exec_time_ns = res.exec_time_ns
    kernel.last_results = res
    return out


# revision 5
# speedup vs baseline: 1.0215x; 1.0215x over previous
"""Cached single-head attention (B=4, QLEN=PAST=2048, D=2048) on 8 Trainium2
NeuronCores — fp8-DoubleRow variant (final).

Sharding: identical to baseline (core (b, h) owns 2048 KV positions: past half
h plus the new keys projected from query half h; 2-core AllGather exchanges
Q^T halves; host combines partial softmax numer/denom).

New in v2: the attention-phase matmuls use fp8 DoubleRow (2 MACs/cell/cycle)
where precision allows:
  - scores vs NEW KV (projected k, sigma~0.58) in fp8-DR; scores vs PAST KV
    (raw N(0,1) k) stay bf16.
  - P@V: new-KV chunks in fp8-DR with mean-centered P (dp = p - c) and an
    exact host-side rank-1 correction c * colsum(v); past-KV chunks bf16.
The centering suppresses fp8 quantization error ~3x and the exact correction
removes the common-mode error of fp8(v).
"""

import sys

sys.path.insert(0, "/opt/trn_rl_repo")

import numpy as np
import ml_dtypes

import concourse.bacc as bacc
import concourse.mybir as mybir
import concourse.tile as tile
from concourse.bass_utils import run_bass_kernel_spmd
from concourse.tile_rust import add_dep_helper

BF16 = mybir.dt.bfloat16
F32 = mybir.dt.float32
DR = mybir.MatmulPerfMode.DoubleRow

# fp8 flavor for the DoubleRow matmuls ('e4' always legal; 'e3' pending HW
# validation of the 4-bit mantissa surviving the DR datapath)
F8_KIND = "e4"
F8 = mybir.dt.float8e4 if F8_KIND == "e4" else mybir.dt.float8e3
F8NP = ml_dtypes.float8_e4m3 if F8_KIND == "e4" else ml_dtypes.float8_e3m4
if F8_KIND == "e3":
    mybir.MATMUL_PERF_MODE_DTYPES = tuple(mybir.MATMUL_PERF_MODE_DTYPES) + (F8,)

B = 4
T = 2048  # QLEN == PAST
D = 2048
P = 128
H = T // 2  # query/kv half owned by one core
DC = D // P  # 16 contraction chunks
EC = D // P  # 16 e-chunks
KC = 16  # kv chunks of 128 (2048 kv positions per core)
PKC = 8  # past kv chunks (0..7); new are 8..15
QBS = 512  # q block size
NQB = T // QBS  # 4 q blocks
WEB = 256  # weight tile e-block width
NWB = D // WEB  # 8 weight tiles per W
SCALE = 1.0 / float(np.sqrt(D))
# centering constants for P = exp(scale * scores): E[p] per kv half
C_PAST = 1.33
C_NEW = 1.06
# clamp so dp fits fp8 range (needed for e3m4 max 15.5; harmless for e4m3)
DP_CLAMP = 15.0 if F8_KIND == "e3" else 200.0

_NC_CACHE: dict = {}


def build_nc():
    nc = bacc.Bacc()
    xa = nc.dram_tensor("xa", [P, DC, QBS], BF16, kind="ExternalInput")
    xb = nc.dram_tensor("xb", [P, DC, QBS], BF16, kind="ExternalInput")
    wq = nc.dram_tensor("wq", [NWB, P, DC, WEB], BF16, kind="ExternalInput")
    wk = nc.dram_tensor("wk", [NWB, P, DC, WEB], BF16, kind="ExternalInput")
    xa8 = nc.dram_tensor("xa8", [P, DC, QBS], F8, kind="ExternalInput")
    xb8 = nc.dram_tensor("xb8", [P, DC, QBS], F8, kind="ExternalInput")
    wv8 = nc.dram_tensor("wv8", [NWB, P, DC, WEB], F8, kind="ExternalInput")
    pk = nc.dram_tensor("pk", [P, EC, H], BF16, kind="ExternalInput")
    pv = nc.dram_tensor("pv", [P, PKC, D], BF16, kind="ExternalInput")
    numer = nc.dram_tensor("numer", [T, D], F32, kind="ExternalOutput")
    denom = nc.dram_tensor("denom", [1, T], F32, kind="ExternalOutput")

    with tile.TileContext(nc) as tc:
        _emit(nc, tc, xa, xb, xa8, xb8, wq, wk, wv8, pk, pv, numer, denom)
    nc.finalize()
    return nc


def _emit(nc, tc, xa_d, xb_d, xa8_d, xb8_d, wq_d, wk_d, wv8_d, pk_d, pv_d, numer, denom):
    with (
        tc.tile_pool(name="res", bufs=1) as res,
        tc.tile_pool(name="dram", bufs=1, space="DRAM") as dram,
    ):
        # Past K^T bf16: kt_bf[p, ec, kv] (kv 0:1024). New K^T fp8: kt8[p, ec, n]
        # (new kv 0:1024 -> chunks 8..15 of the core's kv range).
        kt_bf = res.tile([P, EC, H], BF16)
        kt8 = res.tile([P, EC, H], F8)
        # V: past chunks bf16 v_bf[p, c, e] (c=0..7), halves in fp8 v8[p, c, e]
        # covering the kv chunks that run through the fp8 PV path.  Past fp8
        # copy comes from host (pv8), new half is cast from the V-proj PSUM.
        v_bf = res.tile([P, PKC, D], BF16)
        v8 = res.tile([P, KC - PKC, D], F8)
        qt0 = res.tile([P, EC, QBS], BF16)  # persistent qb=0 prefetch
        ones = nc.const_aps.tensor(1.0, (P, 1), BF16)
        # DoubleRow weight APs need the plane-pair step to be a multiple of
        # 16B; pad the ones column out to 16 and slice [:, :, 0:1].
        ones8 = res.tile([P, 2, 16], F8)
        nc.vector.memset(ones8[:], 1.0)
        qtd_own = dram.tile([H // QBS, EC, P, QBS], BF16)
        qtd_full = dram.tile([2, H // QBS, EC, P, QBS], BF16)

        # ---- PE warmup: dummy matmuls with no data deps run during the
        # initial DMA wait so the HAM clock gate reaches 8/8 before the first
        # real matmul (otherwise the first ~3.4us of matmuls run at 1.2 GHz).
        with tc.tile_pool(name="wps", bufs=1, space="PSUM") as wps_pool:
            wps = wps_pool.tile([P, QBS], F32, tag="warm")
            for _ in range(150):
                nc.tensor.matmul(wps[0:1, 0:1], ones[:], ones[:], start=True, stop=True)

        # ---- prologue: projections (bf16, unchanged from baseline) ----
        with (
            tc.tile_pool(name="xhp", bufs=1) as xh_pool,
            tc.tile_pool(name="w", bufs=3) as w_pool,
            tc.tile_pool(name="qstage", bufs=2) as qstage,
            tc.tile_pool(name="pps", bufs=4, space="PSUM") as pps,
        ):
            xh = xh_pool.tile([P, 2, DC, QBS], BF16, tag="xh")
            xh8 = xh_pool.tile([P, 2, DC, QBS], F8, tag="xh8")
            wv0 = xh_pool.tile([P, DC, WEB], F8, tag="wv0")

            anchor = None
            for eb in range(NWB):
                wq = w_pool.tile([P, DC, WEB], BF16, tag="w")
                nc.sync.dma_start(wq[:], wq_d[eb])
                if eb == 0:
                    nc.sync.dma_start(xh[:, 0, 0 : DC // 2], xa_d[:, 0 : DC // 2])
                    nc.sync.dma_start(xh[:, 0, DC // 2 : DC], xa_d[:, DC // 2 : DC])
                    nc.sync.dma_start(xh[:, 1], xb_d[:])
                for es in range(WEB // P):
                    ec = (eb * WEB) // P + es
                    for qb in range(H // QBS):
                        ps = pps.tile([P, QBS], F32, tag="proj")
                        for dc in range(DC):
                            nc.tensor.matmul(
                                ps[:],
                                wq[:, dc, es * P : (es + 1) * P],
                                xh[:, qb, dc, :],
                                start=(dc == 0),
                                stop=(dc == DC - 1),
                            )
                        qs = qstage.tile([P, QBS], BF16, tag="qs")
                        cp = nc.vector.tensor_copy(qs[:], ps[:])
                        if anchor is None:
                            anchor = cp
                        nc.sync.dma_start(qtd_own[qb, ec], qs[:])

            # bulk past-K/V loads, gated behind the first Q^T tile
            first = None
            for c in range(NWB):
                ktd = nc.sync.dma_start(
                    kt_bf[:, 2 * c : 2 * c + 2, :], pk_d[:, 2 * c : 2 * c + 2, :]
                )
                if first is None:
                    first = ktd
                nc.sync.dma_start(v_bf[:, c, :], pv_d[:, c, :])
            add_dep_helper(anchor.ins, first.ins, reason="delay bulk past load")
            nc.sync.dma_start(xh8[:, 0], xa8_d[:])
            nc.sync.dma_start(xh8[:, 1], xb8_d[:])
            nc.sync.dma_start(wv0[:], wv8_d[0])

            nc.gpsimd.collective_compute(
                "AllGather",
                mybir.AluOpType.bypass,
                replica_groups=[[0, 1], [2, 3], [4, 5], [6, 7]],
                ins=[qtd_own.opt()],
                outs=[qtd_full.opt()],
            )

            # K_new^T -> kt8 (fp8)
            for eb in range(NWB):
                wk = w_pool.tile([P, DC, WEB], BF16, tag="w")
                nc.sync.dma_start(wk[:], wk_d[eb])
                for es in range(WEB // P):
                    ec = (eb * WEB) // P + es
                    for nb in range(H // QBS):
                        ps = pps.tile([P, QBS], F32, tag="proj")
                        for dc in range(DC):
                            nc.tensor.matmul(
                                ps[:],
                                wk[:, dc, es * P : (es + 1) * P],
                                xh[:, nb, dc, :],
                                start=(dc == 0),
                                stop=(dc == DC - 1),
                            )
                        nc.vector.tensor_copy(
                            kt8[:, ec, nb * QBS : (nb + 1) * QBS], ps[:]
                        )
            # V_new -> v8: fp8-DR (x8 @ (16*Wv)8), copy-out scales by 1/16.
            # v errors are suppressed by the centered-PV host correction.
            for eb in range(NWB):
                if eb == 0:
                    wv = wv0
                else:
                    wv = w_pool.tile([P, DC, WEB], F8, tag="w8")
                    nc.sync.dma_start(wv[:], wv8_d[eb])
                for tch in range(H // P):
                    hf, ts_ = divmod(tch, QBS // P)
                    ps = pps.tile([P, WEB], F32, tag="proj")
                    for dc2 in range(DC // 2):
                        nc.tensor.matmul(
                            ps[:],
                            xh8[:, hf, 2 * dc2 : 2 * dc2 + 2, ts_ * P : (ts_ + 1) * P],
                            wv[:, 2 * dc2 : 2 * dc2 + 2, :],
                            start=(dc2 == 0),
                            stop=(dc2 == DC // 2 - 1),
                            perf_mode=DR,
                        )
                    nc.vector.tensor_scalar(
                        v8[:, (KC - PKC - H // P) + tch, eb * WEB : (eb + 1) * WEB],
                        ps[:],
                        1.0 / 16.0,
                        None,
                        mybir.AluOpType.mult,
                    )
            nc.sync.dma_start(qt0[:], qtd_full[0, 0].rearrange("ec p q -> p ec q"))

        # ---- attention ----
        with (
            tc.tile_pool(name="res2", bufs=1) as res2,
            tc.tile_pool(name="qt", bufs=2) as qt_pool,
            tc.tile_pool(name="qt8p", bufs=2) as qt8_pool,
            tc.tile_pool(name="pt", bufs=1) as pt_pool,
            tc.tile_pool(name="pex", bufs=3) as pex_pool,
            tc.tile_pool(name="ostage", bufs=2) as ostage,
            tc.tile_pool(name="sps", bufs=2, space="PSUM") as sps,
            tc.tile_pool(name="ops", bufs=3, space="PSUM") as ops,
            tc.tile_pool(name="dps", bufs=2, space="PSUM") as dps,
        ):
            denom_sb = res2.tile([1, T], F32, name="denom_sb")
            for qb in range(NQB):
                rank, sub = divmod(qb, NQB // 2)
                if qb == 0:
                    qt = qt0
                else:
                    qt = qt_pool.tile([P, EC, QBS], BF16, tag="qt")
                    nc.sync.dma_start(
                        qt[:], qtd_full[rank, sub].rearrange("ec p q -> p ec q")
                    )
                qt8 = qt8_pool.tile([P, EC, QBS], F8, tag="qt8")
                nc.vector.tensor_copy(qt8[:], qt[:])
                # pt_bf: exp(scores) bf16 for past chunks; dpt8: fp8 centered
                # for new chunks
                pt_bf = pt_pool.tile([P, PKC, QBS], BF16, tag="ptbf")
                dpt8 = pt_pool.tile([P, KC - PKC, QBS], F8, tag="dpt8")
                for kc in range(KC):
                    ps = sps.tile([P, QBS], F32, tag="s")
                    if kc < PKC:
                        for ec in range(EC):
                            nc.tensor.matmul(
                                ps[:],
                                kt_bf[:, ec, kc * P : (kc + 1) * P],
                                qt[:, ec, :],
                                start=(ec == 0),
                                stop=(ec == EC - 1),
                            )
                        nc.scalar.activation(
                            pt_bf[:, kc, :],
                            ps[:],
                            mybir.ActivationFunctionType.Exp,
                            scale=SCALE,
                        )
                    else:
                        for ec2 in range(EC // 2):
                            nc.tensor.matmul(
                                ps[:],
                                kt8[:, 2 * ec2 : 2 * ec2 + 2, (kc - PKC) * P : (kc - PKC + 1) * P],
                                qt8[:, 2 * ec2 : 2 * ec2 + 2, :],
                                start=(ec2 == 0),
                                stop=(ec2 == EC // 2 - 1),
                                perf_mode=DR,
                            )
                        pex = pex_pool.tile([P, QBS], F32, tag="pex")
                        nc.scalar.activation(
                            pex[:],
                            ps[:],
                            mybir.ActivationFunctionType.Exp,
                            scale=SCALE,
                        )
                        nc.vector.tensor_scalar(
                            dpt8[:, kc - PKC, :],
                            pex[:],
                            C_NEW,
                            DP_CLAMP,
                            mybir.AluOpType.subtract,
                            mybir.AluOpType.min,
                        )
                # numer[q, e]: past chunks bf16 + new chunks fp8-DR, one PSUM
                # accumulation group per (qc, eb)
                for qc in range(QBS // P):
                    qrow = qb * (QBS // P) + qc
                    for eb in range(D // QBS):
                        po = ops.tile([P, QBS], F32, tag="o")
                        for kc in range(PKC):
                            nc.tensor.matmul(
                                po[:],
                                pt_bf[:, kc, qc * P : (qc + 1) * P],
                                v_bf[:, kc, eb * QBS : (eb + 1) * QBS],
                                start=(kc == 0),
                                stop=False,
                            )
                        nkc = (KC - PKC) // 2
                        for kc2 in range(nkc):
                            nc.tensor.matmul(
                                po[:],
                                dpt8[:, 2 * kc2 : 2 * kc2 + 2, qc * P : (qc + 1) * P],
                                v8[:, 2 * kc2 : 2 * kc2 + 2, eb * QBS : (eb + 1) * QBS],
                                start=False,
                                stop=(kc2 == nkc - 1),
                                perf_mode=DR,
                            )
                        ost = ostage.tile([P, QBS], F32, tag="ost")
                        nc.vector.tensor_copy(ost[:], po[:])
                        nc.sync.dma_start(
                            numer[
                                qrow * P : (qrow + 1) * P,
                                eb * QBS : (eb + 1) * QBS,
                            ],
                            ost[:],
                        )
                # denom: past via ones@pt_bf (M=1), new via fp8-DR ones
                pd = dps.tile([P, QBS], F32, tag="d")
                for kc in range(PKC):
                    nc.tensor.matmul(
                        pd[0:1, :],
                        ones[:],
                        pt_bf[:, kc, :],
                        start=(kc == 0),
                        stop=False,
                    )
                for kc2 in range((KC - PKC) // 2):
                    nc.tensor.matmul(
                        pd[0:1, :],
                        ones8[:, :, 0:1],
                        dpt8[:, 2 * kc2 : 2 * kc2 + 2, :],
                        start=False,
                        stop=(kc2 == (KC - PKC) // 2 - 1),
                        perf_mode=DR,
                    )
                nc.vector.tensor_copy(
                    denom_sb[:, qb * QBS : (qb + 1) * QBS], pd[0:1, :]
                )
                nc.sync.dma_start(
                    denom[:, qb * QBS : (qb + 1) * QBS],
                    denom_sb[:, qb * QBS : (qb + 1) * QBS],
                )


def _get_nc():
    if "nc" not in _NC_CACHE:
        _NC_CACHE["nc"] = build_nc()
    return _NC_CACHE["nc"]


def _pack_w(W, bf):
    return np.ascontiguousarray(
        np.asarray(W).reshape(NWB, WEB, DC, P).transpose(0, 3, 2, 1)
    ).astype(bf)


def make_in_maps(x, past_k, past_v, Wq, Wk, Wv):
    bf = ml_dtypes.bfloat16
    wq = _pack_w(Wq, bf)
    wk = _pack_w(Wk, bf)
    wv8 = _pack_w(np.asarray(Wv) * np.float32(16.0), F8NP)
    in_maps = []
    for b in range(B):
        for h in range(2):
            sel = slice(H * h, H * (h + 1))
            xs = np.asarray(x[b, sel])  # [H, D]
            xap = np.ascontiguousarray(
                xs[0:QBS].reshape(QBS, DC, P).transpose(2, 1, 0)
            )
            xbp0 = np.ascontiguousarray(
                xs[QBS:H].reshape(QBS, DC, P).transpose(2, 1, 0)
            )
            xa = xap.astype(bf)
            xbp = xbp0.astype(bf)
            xa8p = xap.astype(F8NP)
            xb8p = xbp0.astype(F8NP)
            pk = np.ascontiguousarray(
                np.asarray(past_k[b, sel]).reshape(H, EC, P).transpose(2, 1, 0)
            ).astype(bf)
            # past V chunks: [p, c, e] = past_v[b, hH + c*P + p, e]
            pvr = np.asarray(past_v[b, sel]).reshape(H // P, P, D).transpose(1, 0, 2)
            pvp = np.ascontiguousarray(pvr[:, :PKC]).astype(bf)
            in_maps.append(
                {
                    "xa": xa,
                    "xb": xbp,
                    "xa8": xa8p,
                    "xb8": xb8p,
                    "wq": wq,
                    "wk": wk,
                    "wv8": wv8,
                    "pk": pk,
                    "pv": pvp,
                }
            )
    return in_maps


def combine(results, corr_num, corr_den):
    out = np.empty((B, T, D), dtype=np.float32)
    for b in range(B):
        r0, r1 = results[2 * b], results[2 * b + 1]
        num = r0["numer"].astype(np.float64) + r1["numer"] + corr_num[b]
        den = (
            r0["denom"].astype(np.float64) + r1["denom"]
        ).reshape(T) + corr_den
        out[b] = (num / den[:, None]).astype(np.float32)
    return np.round(out, 4)


def host_corrections(x, past_v, Wv):
    """c * colsum(v) over the fp8 PV chunks (new half of both cores), exact."""
    x64 = np.asarray(x, dtype=np.float64)
    Wv64 = np.asarray(Wv, dtype=np.float64)
    corr_num = np.empty((B, D), np.float64)
    for b in range(B):
        xsum = x64[b].sum(axis=0)  # both halves' new v
        corr_num[b] = C_NEW * (xsum @ Wv64.T)
    corr_den = C_NEW * (2 * H)  # 1024 new kv per core, 2 cores per batch
    return corr_num, corr_den


def kernel(x, past_k, past_v, Wq, Wk, Wv, _trace=False, _trace_cores=None):
    nc = _get_nc()
    in_maps = make_in_maps(x, past_k, past_v, Wq, Wk, Wv)
    corr_num, corr_den = host_corrections(x, past_v, Wv)
    res = run_bass_kernel_spmd(
        nc,
        in_maps,
        list(range(8)),
        trace=_trace,
        trace_cores=_trace_cores,
    )
    out = combine(res.results, corr_num, corr_den)
    kernel.last_exec_time_ns = res.exec_time_ns
    kernel.last_results = res
    return out
